# revision 2
# baseline (speedup 1.0000x reference)
"""Trainium2 Bass kernel for nn_HarmonicOscillatorOrbitals.

Computes out[b, i, j] = exp(-s^2/2) * H_j(s), s = omega * x[b, i, 0],
for j = 0..31 (physicists' Hermite polynomials via the three-term
recurrence), data-parallel over 8 NeuronCores on the leading batch axis.

Device algorithm (per core, 65536/8 = 8192 batches = 262144 scalar
elements viewed as [128 partitions, 2048 free]):
  t   = 2*omega*x                      (ACT, per-partition scale)
  env = exp(-(t/2)^2 / 2)              (ACT Square + Exp)
  G_0 = env, G_1 = t*env               (envelope folded into the
  G_k = t*G_{k-1} - 2(k-1)*G_{k-2}      recurrence: G_k = env*H_k)
G columns are written interleaved into an output tile [128, F*32]
(free = f*32 + j) so each output DMA is one large contiguous block.
The 30x2 tensor-tensor ops per tile are split between the Vector
engine (tensor_mul + fused scalar_tensor_tensor) and GPSIMD.
"""

from contextlib import ExitStack

import numpy as np

import concourse.bacc as bacc
import concourse.mybir as mybir
import concourse.tile as tile
from concourse.bass_utils import run_bass_kernel_spmd

F32 = mybir.dt.float32
AF = mybir.ActivationFunctionType
ALU = mybir.AluOpType

NJ = 32          # number of Hermite orders (= last two dims of x/out)
N_CORES = 8
B = 65536        # full batch
BC = B // N_CORES
E = BC * NJ // 128   # free elems per partition per core = 2048
F = 512              # tile width (free elems per partition per tile)
GPS_KS = frozenset({4, 7, 10, 13, 16, 19, 22, 25, 28})  # ks on GPSIMD


def _build(gps_ks=GPS_KS, tile_f=F):
    nc = bacc.Bacc("TRN2", target_bir_lowering=False, debug=False)
    x_d = nc.dram_tensor("x", [128, E], F32, kind="ExternalInput").ap()
    om_d = nc.dram_tensor("om", [1, 1], F32, kind="ExternalInput").ap()
    out_d = nc.dram_tensor("out", [128, E * NJ], F32, kind="ExternalOutput").ap()

    n_tiles = E // tile_f
    with tile.TileContext(nc) as tc, ExitStack() as ctx:
        const_pool = ctx.enter_context(tc.tile_pool(name="const", bufs=1))
        xp = ctx.enter_context(tc.tile_pool(name="xp", bufs=2))
        qp = ctx.enter_context(tc.tile_pool(name="qp", bufs=3))
        op = ctx.enter_context(tc.tile_pool(name="op", bufs=2))

        om1 = const_pool.tile([128, 1], F32)
        nc.sync.dma_start(om1[0:1, :], om_d[:, :])
        om2 = const_pool.tile([128, 1], F32)
        nc.gpsimd.partition_broadcast(om2[:, :], om1[0:1, :])
        nc.scalar.mul(om2[:, :], om2[:, :], 2.0)  # om2 = 2*omega

        for it in range(n_tiles):
            f0 = it * tile_f
            x_t = xp.tile([128, tile_f], F32)
            nc.sync.dma_start(x_t[:, :], x_d[:, f0 : f0 + tile_f])

            t_t = xp.tile([128, tile_f], F32, tag="t")
            nc.scalar.mul(t_t[:, :], x_t[:, :], om2[:, 0:1])  # t = 2*omega*x

            sq = xp.tile([128, tile_f], F32, tag="sq")
            nc.scalar.activation(sq[:, :], t_t[:, :], AF.Square, scale=0.5)

            o_t = op.tile([128, tile_f * NJ], F32)
            oj = o_t[:, :].rearrange("p (f j) -> p j f", j=NJ)
            nc.scalar.activation(oj[:, 0, :], sq[:, :], AF.Exp, scale=-0.5)
            nc.vector.tensor_mul(oj[:, 1, :], t_t[:, :], oj[:, 0, :])

            for k in range(2, NJ):
                q_t = qp.tile([128, tile_f], F32)
                if k in gps_ks:
                    # GPSIMD lacks the fused scalar_tensor_tensor; ACT
                    # supplies the scaled term instead.
                    r_t = qp.tile([128, tile_f], F32, tag="r")
                    nc.gpsimd.tensor_mul(q_t[:, :], t_t[:, :], oj[:, k - 1, :])
                    nc.scalar.mul(r_t[:, :], oj[:, k - 2, :], 2.0 * (k - 1))
                    nc.gpsimd.tensor_sub(oj[:, k, :], q_t[:, :], r_t[:, :])
                else:
                    nc.vector.tensor_mul(q_t[:, :], t_t[:, :], oj[:, k - 1, :])
                    nc.vector.scalar_tensor_tensor(
                        oj[:, k, :],
                        oj[:, k - 2, :],
                        -2.0 * (k - 1),
                        q_t[:, :],
                        ALU.mult,
                        ALU.add,
                    )

            nc.sync.dma_start(out_d[:, f0 * NJ : (f0 + tile_f) * NJ], o_t[:, :])

    nc.compile()
    return nc


_CACHED_NC = None


def kernel(x: np.ndarray, omega_kernel: np.ndarray, **run_kwargs) -> np.ndarray:
    global _CACHED_NC
    assert x.shape == (B, NJ, 1) and omega_kernel.shape == (1, 1), (
        x.shape,
        omega_kernel.shape,
    )
    x = np.ascontiguousarray(x, np.float32)
    om = np.ascontiguousarray(omega_kernel, np.float32)

    if _CACHED_NC is None:
        _CACHED_NC = _build()
    nc = _CACHED_NC

    in_maps = [
        {
            "x": x[c * BC : (c + 1) * BC].reshape(128, E),
            "om": om,
        }
        for c in range(N_CORES)
    ]
    res = run_bass_kernel_spmd(nc, in_maps, core_ids=list(range(N_CORES)), **run_kwargs)
    outs = [
        np.asarray(r["out"]).reshape(BC, NJ, NJ) for r in res.results
    ]
    full = np.concatenate(outs, axis=0)
    if run_kwargs:
        return full, res
    return full


# revision 12
# speedup vs baseline: 1.2973x; 1.2973x over previous
"""Trainium2 Bass kernel for nn_HarmonicOscillatorOrbitals.

out[b, i, j] = exp(-s^2/2) * H_j(s), s = omega * x[b, i, 0], j = 0..31
(physicists' Hermite polynomials), data-parallel over 8 NeuronCores on
the leading batch axis.

Per core (8192 batches = 262144 scalars as [128 partitions, E=2048]):
  t   = 2*omega*x
  env = exp(-s^2/2) = 2^(t^2 * -log2(e)/8), computed exactly on DVE:
        2^n by float-magic + integer exponent shift, 2^f by a degree-5
        polynomial (fused scalar_tensor_tensor Horner chain) — the ACT
        spline Exp is ~1e-5 off, this path is ~1e-6.
  G_0 = env, G_1 = t*env, G_k = t*G_{k-1} - 2(k-1)*G_{k-2}  (= env*H_k)

The per-element recurrence is serial in k, so elements are split into
two independent column slices: DVE runs one chain (tensor_mul + fused
scalar_tensor_tensor), GPSIMD the other (tensor_mul + tensor_sub, with
ACT supplying the 2(k-1)*G_{k-2} scale-copies two steps ahead). Each
chain keeps its engine fully busy with no cross-engine ping-pong.

G_k slices stay contiguous in SBUF; DRAM output is k-major
[128, 32, E] (1.3KB DMA descriptors) and the host permutes to
(batch, i, j) while unsharding.
"""

from contextlib import ExitStack

import numpy as np

import concourse.bacc as bacc
import concourse.mybir as mybir
import concourse.tile as tile
from concourse.bass_utils import run_bass_kernel_spmd

F32 = mybir.dt.float32
I32 = mybir.dt.int32
AF = mybir.ActivationFunctionType
ALU = mybir.AluOpType

NJ = 32          # number of Hermite orders
N_CORES = 8
B = 65536        # full batch
BC = B // N_CORES
E = BC * NJ // 128   # 2048 free elems per partition per core

TILE_F = 512     # columns per tile
FD = 300         # DVE-owned columns per tile (rest on GPSIMD)

# exp2: env = 2^v, v = sq * K4 with sq = t^2 = 4 s^2
K4 = float(np.float32(-np.log2(np.e) / 8.0))
MAGIC = float(np.float32(1.5 * 2**23))
EXP_B4, EXP_B3, EXP_B2, EXP_B1 = 7.292242, 41.85769, 181.15059, 522.6992
EXP_A5, EXP_A0 = 0.0013260915, 1.0
EXP_EIMM = 127 - 0x4B400000  # (bits(w) + EXP_EIMM) << 23 == bits(2^n)


def _build(e=E, tile_f=TILE_F, fd=FD, accurate_env=False):
    nc = bacc.Bacc("TRN2", target_bir_lowering=False, debug=False)
    x_d = nc.dram_tensor("x", [128, e], F32, kind="ExternalInput").ap()
    om_d = nc.dram_tensor("om", [1, 1], F32, kind="ExternalInput").ap()
    # raw tile dump: per f-tile, the DVE-slice tile [128, NJ*fd] then the
    # GPSIMD-slice tile [128, NJ*fg], verbatim — host unscrambles
    out_d = nc.dram_tensor("out", [128, NJ * e], F32, kind="ExternalOutput").ap()

    fg = tile_f - fd
    n_tiles = e // tile_f
    with tile.TileContext(nc) as tc, ExitStack() as ctx:
        cpool = ctx.enter_context(tc.tile_pool(name="const", bufs=1))
        xp = ctx.enter_context(tc.tile_pool(name="xp", bufs=2))
        ep = ctx.enter_context(tc.tile_pool(name="ep", bufs=2))
        qd = ctx.enter_context(tc.tile_pool(name="qd", bufs=3))
        qg = ctx.enter_context(tc.tile_pool(name="qg", bufs=3))
        gdp = ctx.enter_context(tc.tile_pool(name="gdp", bufs=2))
        ggp = ctx.enter_context(tc.tile_pool(name="ggp", bufs=2))

        om1 = cpool.tile([128, 1], F32)
        nc.sync.dma_start(om1[0:1, :], om_d[:, :])
        om2 = cpool.tile([128, 1], F32)
        nc.gpsimd.partition_broadcast(om2[:, :], om1[0:1, :])
        nc.scalar.mul(om2[:, :], om2[:, :], 2.0)  # om2 = 2*omega

        # int32 constants for the exponent fixup: (bits(w) + EXP_EIMM) << 23
        addc = cpool.tile([128, tile_f], I32)
        nc.vector.memset(addc[:, :], EXP_EIMM)
        t23 = cpool.tile([128, tile_f], I32)
        nc.vector.memset(t23[:, :], 23)

        for it in range(n_tiles):
            f0 = it * tile_f
            x_t = xp.tile([128, tile_f], F32)
            nc.sync.dma_start(x_t[:, :], x_d[:, f0 : f0 + tile_f])
            t_t = xp.tile([128, tile_f], F32, tag="t")
            nc.scalar.mul(t_t[:, :], x_t[:, :], om2[:, 0:1])  # t = 2*omega*x

            gd_t = gdp.tile([128, NJ * fd], F32)
            gdj = gd_t[:, :].rearrange("p (k f) -> p k f", k=NJ)
            gg_t = ggp.tile([128, NJ * fg], F32)
            ggj = gg_t[:, :].rearrange("p (k f) -> p k f", k=NJ)

            if accurate_env:
                # ---- exact exp2 on DVE, full tile width ----
                sq = ep.tile([128, tile_f], F32, tag="sq")
                nc.vector.tensor_mul(sq[:, :], t_t[:, :], t_t[:, :])
                v_t = ep.tile([128, tile_f], F32, tag="v")
                nc.vector.tensor_scalar_mul(v_t[:, :], sq[:, :], K4)
                w_t = ep.tile([128, tile_f], F32, tag="w")
                nc.vector.tensor_scalar_add(w_t[:, :], v_t[:, :], MAGIC)
                n_t = ep.tile([128, tile_f], F32, tag="n")
                nc.vector.tensor_scalar_sub(n_t[:, :], w_t[:, :], MAGIC)
                f_t = ep.tile([128, tile_f], F32, tag="f")
                nc.vector.tensor_sub(f_t[:, :], v_t[:, :], n_t[:, :])
                p_t = ep.tile([128, tile_f], F32, tag="p")
                nc.vector.scalar_tensor_tensor(
                    p_t[:, :], f_t[:, :], EXP_B4, f_t[:, :], ALU.add, ALU.mult
                )
                for bb in (EXP_B3, EXP_B2, EXP_B1):
                    nc.vector.scalar_tensor_tensor(
                        p_t[:, :], p_t[:, :], bb, f_t[:, :], ALU.add, ALU.mult
                    )
                nc.vector.tensor_scalar(
                    p_t[:, :], p_t[:, :], EXP_A5, EXP_A0, ALU.mult, ALU.add
                )
                e2_t = ep.tile([128, tile_f], I32, tag="e2")
                nc.vector.tensor_tensor(
                    e2_t[:, :], w_t[:, :].bitcast(I32), addc[:, :], ALU.add
                )
                nc.vector.tensor_tensor(
                    e2_t[:, :], e2_t[:, :], t23[:, :], ALU.logical_shift_left
                )
                e2f = e2_t[:, :].bitcast(F32)
                # env split straight into the two G tiles (k = 0)
                nc.vector.tensor_mul(gdj[:, 0, :], p_t[:, 0:fd], e2f[:, 0:fd])
                nc.vector.tensor_mul(ggj[:, 0, :], p_t[:, fd:], e2f[:, fd:])
            else:
                sq = ep.tile([128, tile_f], F32, tag="sq")
                nc.scalar.activation(sq[:, :], t_t[:, :], AF.Square, scale=0.5)
                nc.scalar.activation(gdj[:, 0, :], sq[:, 0:fd], AF.Exp, scale=-0.5)
                nc.scalar.activation(ggj[:, 0, :], sq[:, fd:], AF.Exp, scale=-0.5)

            # G_1 = t * env, each engine seeds its own chain
            nc.vector.tensor_mul(gdj[:, 1, :], t_t[:, 0:fd], gdj[:, 0, :])
            nc.gpsimd.tensor_mul(ggj[:, 1, :], t_t[:, fd:], ggj[:, 0, :])

            for k in range(2, NJ):
                c = 2.0 * (k - 1)
                # DVE chain
                q_t = qd.tile([128, fd], F32)
                nc.vector.tensor_mul(q_t[:, :], t_t[:, 0:fd], gdj[:, k - 1, :])
                nc.vector.scalar_tensor_tensor(
                    gdj[:, k, :], gdj[:, k - 2, :], -c, q_t[:, :], ALU.mult, ALU.add
                )
                # GPSIMD chain (ACT supplies c*G_{k-2})
                qg_t = qg.tile([128, fg], F32)
                rg_t = qg.tile([128, fg], F32, tag="rg")
                nc.gpsimd.tensor_mul(qg_t[:, :], t_t[:, fd:], ggj[:, k - 1, :])
                nc.scalar.mul(rg_t[:, :], ggj[:, k - 2, :], c)
                nc.gpsimd.tensor_sub(ggj[:, k, :], qg_t[:, :], rg_t[:, :])

            base = it * NJ * tile_f
            nc.sync.dma_start(out_d[:, base : base + NJ * fd], gd_t[:, :])
            nc.sync.dma_start(
                out_d[:, base + NJ * fd : base + NJ * tile_f], gg_t[:, :]
            )

    nc.compile()
    return nc


_CACHED_NC = None


def kernel(x: np.ndarray, omega_kernel: np.ndarray, **run_kwargs) -> np.ndarray:
    global _CACHED_NC
    assert x.shape == (B, NJ, 1) and omega_kernel.shape == (1, 1), (
        x.shape,
        omega_kernel.shape,
    )
    x = np.ascontiguousarray(x, np.float32)
    om = np.ascontiguousarray(omega_kernel, np.float32)

    if _CACHED_NC is None:
        _CACHED_NC = _build()
    nc = _CACHED_NC

    in_maps = [
        {
            "x": x[c * BC : (c + 1) * BC].reshape(128, E),
            "om": om,
        }
        for c in range(N_CORES)
    ]
    res = run_bass_kernel_spmd(nc, in_maps, core_ids=list(range(N_CORES)), **run_kwargs)
    fg = TILE_F - FD
    full = np.empty((B, NJ, NJ), np.float32)
    for c in range(N_CORES):
        arr = np.asarray(res.results[c]["out"]).reshape(128, NJ * E)
        out3 = np.empty((128, NJ, E), np.float32)
        for it in range(E // TILE_F):
            f0, base = it * TILE_F, it * NJ * TILE_F
            out3[:, :, f0 : f0 + FD] = arr[
                :, base : base + NJ * FD
            ].reshape(128, NJ, FD)
            out3[:, :, f0 + FD : f0 + TILE_F] = arr[
                :, base + NJ * FD : base + NJ * TILE_F
            ].reshape(128, NJ, fg)
        full[c * BC : (c + 1) * BC] = out3.transpose(0, 2, 1).reshape(BC, NJ, NJ)
    if run_kwargs:
        return full, res
    return full


# revision 15
# speedup vs baseline: 1.3909x; 1.0722x over previous
"""Trainium2 Bass kernel for nn_HarmonicOscillatorOrbitals.

out[b, i, j] = exp(-s^2/2) * H_j(s), s = omega * x[b, i, 0], j = 0..31
(physicists' Hermite polynomials), data-parallel over 8 NeuronCores on
the leading batch axis.

Per core (8192 batches = 262144 scalars as [128 partitions, E=2048]):
  t   = 2*omega*x
  env = exp(-s^2/2) = 2^(t^2 * -log2(e)/8), computed exactly on DVE:
        2^n by float-magic + integer exponent shift, 2^f by a degree-5
        polynomial (fused scalar_tensor_tensor Horner chain) — the ACT
        spline Exp is ~1e-5 off, this path is ~1e-6.
  G_0 = env, G_1 = t*env, G_k = t*G_{k-1} - 2(k-1)*G_{k-2}  (= env*H_k)

The per-element recurrence is serial in k, so elements are split into
two independent column slices: DVE runs one chain (tensor_mul + fused
scalar_tensor_tensor), GPSIMD the other (tensor_mul + tensor_sub, with
ACT supplying the 2(k-1)*G_{k-2} scale-copies two steps ahead). Each
chain keeps its engine fully busy with no cross-engine ping-pong.

G_k slices stay contiguous in SBUF; DRAM output is k-major
[128, 32, E] (1.3KB DMA descriptors) and the host permutes to
(batch, i, j) while unsharding.
"""

from contextlib import ExitStack

import numpy as np

import concourse.bacc as bacc
import concourse.mybir as mybir
import concourse.tile as tile
from concourse.bass_utils import run_bass_kernel_spmd

F32 = mybir.dt.float32
I32 = mybir.dt.int32
AF = mybir.ActivationFunctionType
ALU = mybir.AluOpType

NJ = 32          # number of Hermite orders
N_CORES = 8
B = 65536        # full batch
BC = B // N_CORES
E = BC * NJ // 128   # 2048 free elems per partition per core

TILE_F = 512     # columns per tile
FD = 300         # DVE-owned columns per tile (rest on GPSIMD)

# exp2: env = 2^v, v = sq * K4 with sq = t^2 = 4 s^2
K4 = float(np.float32(-np.log2(np.e) / 8.0))
MAGIC = float(np.float32(1.5 * 2**23))
EXP_B4, EXP_B3, EXP_B2, EXP_B1 = 7.292242, 41.85769, 181.15059, 522.6992
EXP_A5, EXP_A0 = 0.0013260915, 1.0
EXP_EIMM = 127 - 0x4B400000  # (bits(w) + EXP_EIMM) << 23 == bits(2^n)


def _build(e=E, tile_f=TILE_F, fd=FD, accurate_env=False):
    nc = bacc.Bacc("TRN2", target_bir_lowering=False, debug=False)
    x_d = nc.dram_tensor("x", [128, e], F32, kind="ExternalInput").ap()
    om_d = nc.dram_tensor("om", [1, 1], F32, kind="ExternalInput").ap()
    # raw tile dump: per f-tile, the DVE-slice tile [128, NJ*fd] then the
    # GPSIMD-slice tile [128, NJ*fg], verbatim — host unscrambles
    out_d = nc.dram_tensor("out", [128, NJ * e], F32, kind="ExternalOutput").ap()

    fg = tile_f - fd
    n_tiles = e // tile_f
    with tile.TileContext(nc) as tc, ExitStack() as ctx:
        cpool = ctx.enter_context(tc.tile_pool(name="const", bufs=1))
        xp = ctx.enter_context(tc.tile_pool(name="xp", bufs=2))
        ep = ctx.enter_context(tc.tile_pool(name="ep", bufs=2))
        qd = ctx.enter_context(tc.tile_pool(name="qd", bufs=3))
        qg = ctx.enter_context(tc.tile_pool(name="qg", bufs=3))
        gdp = ctx.enter_context(tc.tile_pool(name="gdp", bufs=2))
        ggp = ctx.enter_context(tc.tile_pool(name="ggp", bufs=2))

        om1 = cpool.tile([128, 1], F32)
        nc.sync.dma_start(om1[0:1, :], om_d[:, :])
        om2 = cpool.tile([128, 1], F32)
        nc.gpsimd.partition_broadcast(om2[:, :], om1[0:1, :])
        nc.scalar.mul(om2[:, :], om2[:, :], 2.0)  # om2 = 2*omega

        # int32 constants for the exponent fixup: (bits(w) + EXP_EIMM) << 23
        addc = cpool.tile([128, tile_f], I32)
        nc.vector.memset(addc[:, :], EXP_EIMM)
        t23 = cpool.tile([128, tile_f], I32)
        nc.vector.memset(t23[:, :], 23)

        for it in range(n_tiles):
            f0 = it * tile_f
            x_t = xp.tile([128, tile_f], F32)
            nc.sync.dma_start(x_t[:, :], x_d[:, f0 : f0 + tile_f])
            t_t = xp.tile([128, tile_f], F32, tag="t")
            nc.scalar.mul(t_t[:, :], x_t[:, :], om2[:, 0:1])  # t = 2*omega*x

            # four k-quarters per slice: DMA each out as soon as its 8
            # columns are done, so pool slots recycle at 1/4-tile grain
            gd_q = [
                gdp.tile([128, 8 * fd], F32, name=f"gdq{q}_{it}", tag=f"gd{q}")
                for q in range(4)
            ]
            gg_q = [
                ggp.tile([128, 8 * fg], F32, name=f"ggq{q}_{it}", tag=f"gg{q}")
                for q in range(4)
            ]

            def gds(k):
                return gd_q[k // 8][:, (k % 8) * fd : (k % 8 + 1) * fd]

            def ggs(k):
                return gg_q[k // 8][:, (k % 8) * fg : (k % 8 + 1) * fg]

            base = it * NJ * tile_f

            def flush_quarter(q):
                nc.sync.dma_start(
                    out_d[:, base + q * 8 * fd : base + (q + 1) * 8 * fd],
                    gd_q[q][:, :],
                )
                goff = base + NJ * fd
                nc.sync.dma_start(
                    out_d[:, goff + q * 8 * fg : goff + (q + 1) * 8 * fg],
                    gg_q[q][:, :],
                )

            if accurate_env:
                # ---- exact exp2 on DVE, full tile width ----
                sq = ep.tile([128, tile_f], F32, tag="sq")
                nc.vector.tensor_mul(sq[:, :], t_t[:, :], t_t[:, :])
                v_t = ep.tile([128, tile_f], F32, tag="v")
                nc.vector.tensor_scalar_mul(v_t[:, :], sq[:, :], K4)
                w_t = ep.tile([128, tile_f], F32, tag="w")
                nc.vector.tensor_scalar_add(w_t[:, :], v_t[:, :], MAGIC)
                n_t = ep.tile([128, tile_f], F32, tag="n")
                nc.vector.tensor_scalar_sub(n_t[:, :], w_t[:, :], MAGIC)
                f_t = ep.tile([128, tile_f], F32, tag="f")
                nc.vector.tensor_sub(f_t[:, :], v_t[:, :], n_t[:, :])
                p_t = ep.tile([128, tile_f], F32, tag="p")
                nc.vector.scalar_tensor_tensor(
                    p_t[:, :], f_t[:, :], EXP_B4, f_t[:, :], ALU.add, ALU.mult
                )
                for bb in (EXP_B3, EXP_B2, EXP_B1):
                    nc.vector.scalar_tensor_tensor(
                        p_t[:, :], p_t[:, :], bb, f_t[:, :], ALU.add, ALU.mult
                    )
                nc.vector.tensor_scalar(
                    p_t[:, :], p_t[:, :], EXP_A5, EXP_A0, ALU.mult, ALU.add
                )
                e2_t = ep.tile([128, tile_f], I32, tag="e2")
                nc.vector.tensor_tensor(
                    e2_t[:, :], w_t[:, :].bitcast(I32), addc[:, :], ALU.add
                )
                nc.vector.tensor_tensor(
                    e2_t[:, :], e2_t[:, :], t23[:, :], ALU.logical_shift_left
                )
                e2f = e2_t[:, :].bitcast(F32)
                # env split straight into the two G tiles (k = 0)
                nc.vector.tensor_mul(gds(0), p_t[:, 0:fd], e2f[:, 0:fd])
                nc.vector.tensor_mul(ggs(0), p_t[:, fd:], e2f[:, fd:])
            else:
                sq = ep.tile([128, tile_f], F32, tag="sq")
                nc.scalar.activation(sq[:, :], t_t[:, :], AF.Square, scale=0.5)
                nc.scalar.activation(gds(0), sq[:, 0:fd], AF.Exp, scale=-0.5)
                nc.scalar.activation(ggs(0), sq[:, fd:], AF.Exp, scale=-0.5)

            # G_1 = t * env, each engine seeds its own chain
            nc.vector.tensor_mul(gds(1), t_t[:, 0:fd], gds(0))
            nc.gpsimd.tensor_mul(ggs(1), t_t[:, fd:], ggs(0))

            for k in range(2, NJ):
                c = 2.0 * (k - 1)
                # DVE chain
                q_t = qd.tile([128, fd], F32)
                nc.vector.tensor_mul(q_t[:, :], t_t[:, 0:fd], gds(k - 1))
                nc.vector.scalar_tensor_tensor(
                    gds(k), gds(k - 2), -c, q_t[:, :], ALU.mult, ALU.add
                )
                # GPSIMD chain (ACT supplies c*G_{k-2})
                qg_t = qg.tile([128, fg], F32)
                rg_t = qg.tile([128, fg], F32, tag="rg")
                nc.gpsimd.tensor_mul(qg_t[:, :], t_t[:, fd:], ggs(k - 1))
                nc.scalar.mul(rg_t[:, :], ggs(k - 2), c)
                nc.gpsimd.tensor_sub(ggs(k), qg_t[:, :], rg_t[:, :])
                if k % 8 == 7:
                    flush_quarter(k // 8)

    nc.compile()
    return nc


_CACHED_NC = None


def kernel(x: np.ndarray, omega_kernel: np.ndarray, **run_kwargs) -> np.ndarray:
    global _CACHED_NC
    assert x.shape == (B, NJ, 1) and omega_kernel.shape == (1, 1), (
        x.shape,
        omega_kernel.shape,
    )
    x = np.ascontiguousarray(x, np.float32)
    om = np.ascontiguousarray(omega_kernel, np.float32)

    if _CACHED_NC is None:
        _CACHED_NC = _build()
    nc = _CACHED_NC

    in_maps = [
        {
            "x": x[c * BC : (c + 1) * BC].reshape(128, E),
            "om": om,
        }
        for c in range(N_CORES)
    ]
    res = run_bass_kernel_spmd(nc, in_maps, core_ids=list(range(N_CORES)), **run_kwargs)
    fg = TILE_F - FD
    full = np.empty((B, NJ, NJ), np.float32)
    for c in range(N_CORES):
        arr = np.asarray(res.results[c]["out"]).reshape(128, NJ * E)
        out3 = np.empty((128, NJ, E), np.float32)
        for it in range(E // TILE_F):
            f0, base = it * TILE_F, it * NJ * TILE_F
            out3[:, :, f0 : f0 + FD] = arr[
                :, base : base + NJ * FD
            ].reshape(128, NJ, FD)
            out3[:, :, f0 + FD : f0 + TILE_F] = arr[
                :, base + NJ * FD : base + NJ * TILE_F
            ].reshape(128, NJ, fg)
        full[c * BC : (c + 1) * BC] = out3.transpose(0, 2, 1).reshape(BC, NJ, NJ)
    if run_kwargs:
        return full, res
    return full
